# revision 1
# baseline (speedup 1.0000x reference)
"""Trainium2 Bass kernel for ExactSequenceAttention.

Reference math (B=4, N=2048, DIM=2048, H=1, hd=2048, S=2048):
    qkv = x @ qkv_w.T + qkv_b -> q, k, v
    attn = softmax(q @ k.T / sqrt(hd))
    ker = (q @ sp_w.T + sp_b) @ kc_w.T + kc_b
    img = (k @ sp_w.T + sp_b) @ ic_w.T + ic_b
    seqw = softmax((ker @ img.T / sqrt(S)) * mask)
    y = 0.5*(attn + seqw) @ v
    out = y @ proj_w.T + proj_b

Sharding: 8 cores = 4 batches x 2 halves of the sequence. Core 2b+h owns
query rows [h*1024,(h+1)*1024) of batch b and computes k/v/img for the
same row range; halves are exchanged with block-wise pair AllGathers
(replica groups [0,1],[2,3],[4,5],[6,7]) issued as soon as each block is
produced so they hide behind compute. The host folds the two seq
projections into single matmuls (Wker = sp_w.T@kc_w.T etc), pre-scales
k by 1/sqrt(hd) and img by 1/sqrt(S).

On-device layout is fully transposed (features on partitions):
scores are computed as scoresT[m, n] (keys on partitions), exp is taken
without max subtraction (scores are O(1)), softmax denominators come from
a ones-vector matmul, and normalization is folded into the combined
weight matrix P before a single yT/proj matmul chain. No transposes.

All matmuls run in float32r (fp22) at bf16 speed with fp32 PSUM accumulation.
"""
import math
import sys

sys.path.insert(0, "/opt/trn_rl_repo")

import numpy as np

P = 128
FD = 512  # matmul free dim

# full-problem dims
DIM = 2048
B, N = 4, 2048
N_CORES = 8
GROUPS = [[0, 1], [2, 3], [4, 5], [6, 7]]


def build_nc(D=DIM, NQ=N // 2, NM=N, gather=True, repeat=1):
    """Build the SPMD program. D=feature dim, NQ=query rows per core,
    NM=key rows (= full N of one batch)."""
    import concourse.bacc as bacc
    import concourse.mybir as mybir
    import concourse.tile as tile
    from concourse import tile_utils
    from contextlib import ExitStack

    tile_utils.max_sbuf_usage = 204 * 1024  # cayman has 208KB usable/partition

    F32 = mybir.dt.float32
    F32R = mybir.dt.float32r
    BF16 = mybir.dt.bfloat16
    AX = mybir.AluOpType
    EXP = mybir.ActivationFunctionType.Exp

    DT = D // P         # feature-dim tiles
    MT = NM // P        # key-row chunks (full)
    NBL = max(NQ // FD, 1)
    NF = min(NQ, FD)
    DB = D // FD
    # local (per-core) key range
    NMH = NM // 2 if gather else NM
    MTH = NMH // P
    MBH = max(NMH // FD, 1)
    MFB = min(NMH, FD)
    PB = MFB // P       # 128-chunks per block
    LCH = MT // 2 if gather else MT  # chunks per half

    nc = bacc.Bacc("TRN2", target_bir_lowering=False, debug=False,
                   num_devices=N_CORES)

    def din(name, shape):
        return nc.dram_tensor(name, list(shape), F32, kind="ExternalInput")

    if not gather:
        xT = din("xT", (D, NM))    # x[b].T  [c, m] (full)
    xTq = din("xTq", (D, NQ))      # x[b].T own-half cols [c, n]
    # weights pre-tiled on host into strip-major layouts for contiguous DMA
    WqT = din("WqT", (DT, D, P))       # [dt][c][d_in]
    WkTs = din("WkTs", (DT, D, P))     # [dt][c][d_in] (pre-scaled 1/sqrt(hd))
    WvT = din("WvT", (D // FD, D, FD))  # [db][c][d_in]
    Wker = din("Wker", (DT, D, P))     # [st][c][s_in]
    Wimg = nc.dram_tensor("Wimg", [DT, D, P], BF16,
                          kind="ExternalInput")  # bf16 (pre-scaled)
    PwT = din("PwT", (DT, D, P))       # [ct][d][c_in]
    bq_d = din("bq", (P, DT))
    bks_d = din("bks", (P, DT))
    bker_d = din("bker", (P, DT))
    bimg_d = din("bimg", (P, DT))
    pb_d = din("pb", (P, DT))
    BV_d = din("BV", (P, D))
    mask_d = din("maskS", (P, MT))
    ones_d = din("ones", (P, 1))
    ones16_d = nc.dram_tensor("ones16", [P, 1], BF16, kind="ExternalInput")

    outT = nc.dram_tensor("outT", [D, NQ], F32, kind="ExternalOutput")

    def ckload(dst, src_2d, cols, chunks=1):
        """Load a (P, DT, w) feature-major tile in `chunks` DMAs so early
        consumers unblock before the full tile lands."""
        chunks = min(chunks, DT)
        gsz = DT // chunks
        for g in range(chunks):
            nc.sync.dma_start(
                dst[:, g * gsz:(g + 1) * gsz, :],
                src_2d[g * gsz * P:(g + 1) * gsz * P, cols]
                .bitcast(dst.dtype).rearrange("(o p) w -> p o w", p=P))

    with tile.TileContext(nc) as tc:
        with ExitStack() as ctx:
            consts = ctx.enter_context(tc.tile_pool(name="consts", bufs=1))
            dram = ctx.enter_context(
                tc.tile_pool(name="dram", bufs=1, space="DRAM"))

            bq = consts.tile([P, DT], F32)
            bks = consts.tile([P, DT], F32)
            bker = consts.tile([P, DT], F32)
            bimg = consts.tile([P, DT], F32)
            pb = consts.tile([P, DT], F32)
            maskS = consts.tile([P, MT], F32)
            ones = consts.tile([P, 1], F32R)
            ones16 = consts.tile([P, 1], BF16)
            nc.sync.dma_start(bq[:], bq_d[:])
            nc.sync.dma_start(bks[:], bks_d[:])
            nc.sync.dma_start(bker[:], bker_d[:])
            nc.sync.dma_start(bimg[:], bimg_d[:])
            nc.sync.dma_start(pb[:], pb_d[:])
            nc.sync.dma_start(maskS[:], mask_d[:])
            nc.sync.dma_start(ones[:], ones_d[:].bitcast(F32R))
            nc.sync.dma_start(ones16[:], ones16_d[:])

            qT_d = dram.tile([D, NQ], BF16)
            kerT_d = dram.tile([D, NQ], BF16)
            # kTs/imgT: [mb][mi][p(d_in)][do][m_in] -- kA/iA chunks contiguous
            kTs_h = dram.tile([MBH, PB, P, DT, P], BF16)
            imgT_h = dram.tile([MBH, PB, P, DT, P], BF16)
            # v: [mb][do][m_in_block][d_in] -- per-(dt) slices contiguous
            v_h = dram.tile([MBH, DT, MFB, P], F32)
            if gather:
                kTs_g = dram.tile([2, MBH, PB, P, DT, P], BF16)
                imgT_g = dram.tile([2, MBH, PB, P, DT, P], BF16)
                v_g = dram.tile([2, MBH, DT, MFB, P], F32)

            def pair_gather(half_blk, gath_blk):
                nc.gpsimd.collective_compute(
                    "AllGather", mybir.AluOpType.bypass,
                    replica_groups=GROUPS,
                    ins=[half_blk[:]], outs=[gath_blk[:]])

            for _rep in range(repeat):
                xsrc = xTq if gather else xT

                # ======== Stage 1c: kTs half, gathered per block ========
                with ExitStack() as s1:
                    wpool = s1.enter_context(tc.tile_pool(name="wres", bufs=DT))
                    xmp = s1.enter_context(tc.tile_pool(name="xmp", bufs=2))
                    ps1 = s1.enter_context(
                        tc.tile_pool(name="ps1c", bufs=4, space="PSUM"))
                    tmps = s1.enter_context(tc.tile_pool(name="tmps1c", bufs=4))

                    xm0 = xmp.tile([P, DT, MFB], F32R, tag="xm")
                    ckload(xm0, xsrc, slice(0, MFB), chunks=8)
                    wk_strips = []
                    for dt in range(DT):
                        w = wpool.tile([P, DT, P], F32R, tag="wres")
                        ckload(w, WkTs[dt], slice(0, P))
                        wk_strips.append(w)
                    for mb in range(MBH):
                        if mb == 0:
                            xm = xm0
                        else:
                            xm = xmp.tile([P, DT, MFB], F32R, tag="xm")
                            ckload(xm, xsrc, slice(mb * MFB, (mb + 1) * MFB),
                                   chunks=4)
                        for dt in range(DT):
                            ps = ps1.tile([P, MFB], F32, tag="ps1c")
                            for ck in range(DT):
                                nc.tensor.matmul(
                                    ps[:], wk_strips[dt][:, ck, :], xm[:, ck, :],
                                    start=(ck == 0), stop=(ck == DT - 1))
                            t = tmps.tile([P, MFB], BF16, tag="t1c")
                            nc.any.tensor_scalar(
                                out=t[:], in0=ps[:], scalar1=bks[:, dt:dt + 1],
                                scalar2=None, op0=AX.add)
                            nc.sync.dma_start(
                                kTs_h[mb][:, :, dt, :].rearrange(
                                    "mi p m -> p mi m"),
                                t[:].rearrange("p (mi m) -> p mi m", mi=PB))
                    if gather:
                        pair_gather(kTs_h, kTs_g)

                # ======== Stage 1e: imgT half from local kTs ========
                with ExitStack() as s1:
                    wpool = s1.enter_context(tc.tile_pool(name="wres3", bufs=DT))
                    kmp = s1.enter_context(tc.tile_pool(name="kmp", bufs=2))
                    ps1 = s1.enter_context(
                        tc.tile_pool(name="ps1e", bufs=4, space="PSUM"))
                    tmps = s1.enter_context(tc.tile_pool(name="tmps1e", bufs=4))

                    def load_km(km, mb):
                        for mi in range(PB):
                            nc.sync.dma_start(
                                km[:, :, mi * P:(mi + 1) * P], kTs_h[mb][mi])
                    km0 = kmp.tile([P, DT, MFB], BF16, tag="km")
                    load_km(km0, 0)
                    wi_strips = []
                    for st in range(DT):
                        w = wpool.tile([P, DT, P], BF16, tag="wres3")
                        ckload(w, Wimg[st], slice(0, P))
                        wi_strips.append(w)
                    for mb in range(MBH):
                        if mb == 0:
                            km = km0
                        else:
                            km = kmp.tile([P, DT, MFB], BF16, tag="km")
                            load_km(km, mb)
                        for st in range(DT):
                            ps = ps1.tile([P, MFB], F32, tag="ps1e")
                            for ck in range(DT):
                                nc.tensor.matmul(
                                    ps[:], wi_strips[st][:, ck, :], km[:, ck, :],
                                    start=(ck == 0), stop=(ck == DT - 1))
                            t = tmps.tile([P, MFB], BF16, tag="t1e")
                            nc.any.tensor_scalar(
                                out=t[:], in0=ps[:], scalar1=bimg[:, st:st + 1],
                                scalar2=None, op0=AX.add)
                            nc.sync.dma_start(
                                imgT_h[mb][:, :, st, :].rearrange(
                                    "mi p m -> p mi m"),
                                t[:].rearrange("p (mi m) -> p mi m", mi=PB))
                    if gather:
                        pair_gather(imgT_h, imgT_g)

                # ======== Stage 1d: v half, gathered per block ========
                with ExitStack() as s1:
                    wpool = s1.enter_context(tc.tile_pool(name="wres2", bufs=DB))
                    bvp = s1.enter_context(tc.tile_pool(name="bvp", bufs=1))
                    xcp = s1.enter_context(tc.tile_pool(name="xcp", bufs=3))
                    ps1 = s1.enter_context(
                        tc.tile_pool(name="ps1d", bufs=4, space="PSUM"))
                    tmps = s1.enter_context(tc.tile_pool(name="tmps1d", bufs=4))

                    xc0 = xcp.tile([P, DT, P], F32R, tag="xc")
                    ckload(xc0, xsrc, slice(0, P))
                    wv_strips = []
                    for db in range(DB):
                        w = wpool.tile([P, DT, FD], F32R, tag="wres2")
                        ckload(w, WvT[db], slice(0, FD), chunks=4)
                        wv_strips.append(w)
                    BV = bvp.tile([P, D], F32)
                    nc.sync.dma_start(BV[:], BV_d[:])
                    for mb in range(MBH):
                        for mi in range(PB):
                            m = mb * PB + mi
                            if m == 0:
                                xc = xc0
                            else:
                                xc = xcp.tile([P, DT, P], F32R, tag="xc")
                                ckload(xc, xsrc, slice(m * P, (m + 1) * P))
                            for db in range(DB):
                                ps = ps1.tile([P, FD], F32, tag="ps1d")
                                for ck in range(DT):
                                    nc.tensor.matmul(
                                        ps[:], xc[:, ck, :],
                                        wv_strips[db][:, ck, :],
                                        start=(ck == 0), stop=(ck == DT - 1))
                                t = tmps.tile([P, FD], F32, tag="t1d")
                                nc.any.tensor_tensor(
                                    t[:], ps[:], BV[:, db * FD:(db + 1) * FD],
                                    AX.add)
                                FDP = FD // P
                                nc.sync.dma_start(
                                    v_h[mb][db * FDP:(db + 1) * FDP,
                                            mi * P:(mi + 1) * P, :].rearrange(
                                        "o p d -> p o d"),
                                    t[:].rearrange("p (o d) -> p o d", o=FDP))
                    if gather:
                        pair_gather(v_h, v_g)

                # ======== Stage 1a+1b: qT then kerT ========
                with ExitStack() as s1:
                    pq = s1.enter_context(tc.tile_pool(name="pq", bufs=1))
                    strips = s1.enter_context(tc.tile_pool(name="strips", bufs=3))
                    ps1 = s1.enter_context(
                        tc.tile_pool(name="ps1", bufs=4, space="PSUM"))
                    tmps = s1.enter_context(tc.tile_pool(name="tmps", bufs=4))

                    xq = pq.tile([P, DT, NQ], F32R, tag="xq")
                    ckload(xq, xTq, slice(0, NQ), chunks=8)
                    qT_sb = pq.tile([P, DT, NQ], F32R, tag="qT")

                    for dt in range(DT):
                        wq = strips.tile([P, DT, P], F32R, tag="w1")
                        ckload(wq, WqT[dt], slice(0, P))
                        for nb in range(NBL):
                            ps = ps1.tile([P, NF], F32, tag="ps1")
                            for ck in range(DT):
                                nc.tensor.matmul(
                                    ps[:], wq[:, ck, :],
                                    xq[:, ck, nb * NF:(nb + 1) * NF],
                                    start=(ck == 0), stop=(ck == DT - 1))
                            nc.any.tensor_scalar(
                                out=qT_sb[:, dt, nb * NF:(nb + 1) * NF],
                                in0=ps[:], scalar1=bq[:, dt:dt + 1],
                                scalar2=None, op0=AX.add)
                        qc = tmps.tile([P, NQ], BF16, tag="qc")
                        nc.any.tensor_copy(
                            out=qc[:], in_=qT_sb[:, dt, :].bitcast(F32))
                        nc.sync.dma_start(qT_d[dt * P:(dt + 1) * P, :], qc[:])

                    for st in range(DT):
                        wk = strips.tile([P, DT, P], F32R, tag="w1")
                        ckload(wk, Wker[st], slice(0, P))
                        for nb in range(NBL):
                            ps = ps1.tile([P, NF], F32, tag="ps1")
                            for ck in range(DT):
                                nc.tensor.matmul(
                                    ps[:], wk[:, ck, :],
                                    qT_sb[:, ck, nb * NF:(nb + 1) * NF],
                                    start=(ck == 0), stop=(ck == DT - 1))
                            t = tmps.tile([P, NF], BF16, tag="t1")
                            nc.any.tensor_scalar(
                                out=t[:], in0=ps[:], scalar1=bker[:, st:st + 1],
                                scalar2=None, op0=AX.add)
                            nc.sync.dma_start(
                                kerT_d[st * P:(st + 1) * P,
                                       nb * NF:(nb + 1) * NF], t[:])

                # ======== Stage 2 ========
                with ExitStack() as s2:
                    blk = s2.enter_context(tc.tile_pool(name="blk", bufs=1))
                    nin = s2.enter_context(tc.tile_pool(name="nin", bufs=1))
                    stream = s2.enter_context(tc.tile_pool(name="stream", bufs=3))
                    small = s2.enter_context(tc.tile_pool(name="small", bufs=2))
                    tmps = s2.enter_context(tc.tile_pool(name="tmps2", bufs=2))
                    psAS = s2.enter_context(
                        tc.tile_pool(name="psAS", bufs=3, space="PSUM"))
                    psSums = s2.enter_context(
                        tc.tile_pool(name="psSums", bufs=1, space="PSUM"))
                    psYO = s2.enter_context(
                        tc.tile_pool(name="psYO", bufs=3, space="PSUM"))

                    for nb in range(NBL):
                        nsl = slice(nb * NF, (nb + 1) * NF)
                        qTn = nin.tile([P, DT, NF], BF16, tag="qTn")
                        kerTn = nin.tile([P, DT, NF], BF16, tag="kerTn")
                        ckload(qTn, qT_d, nsl, chunks=4)
                        ckload(kerTn, kerT_d, nsl, chunks=4)

                        expA = blk.tile([P, MT, NF], BF16, tag="expA")
                        expS = blk.tile([P, MT, NF], BF16, tag="expS")
                        sumA = psSums.tile([1, NF], F32, tag="sumA")
                        sumS = psSums.tile([1, NF], F32, tag="sumS")

                        def tile_chunk(g_t, h_t, mt):
                            if gather:
                                h, l = divmod(mt, LCH)
                                mb, mi = divmod(l, PB)
                                return g_t[h][mb][mi]
                            mb, mi = divmod(mt, PB)
                            return h_t[mb][mi]

                        # ---- 2a: attn scores + exp + col sums ----
                        for mt in range(MT):
                            kA = stream.tile([P, DT, P], BF16, tag="stm")
                            nc.sync.dma_start(
                                kA[:], tile_chunk(
                                    kTs_g if gather else None, kTs_h, mt))
                            psA = psAS.tile([P, NF], F32, tag="psA")
                            for ck in range(DT):
                                nc.tensor.matmul(
                                    psA[:], kA[:, ck, :], qTn[:, ck, :],
                                    start=(ck == 0), stop=(ck == DT - 1))
                            nc.scalar.activation(expA[:, mt, :], psA[:], EXP)
                            nc.tensor.matmul(
                                sumA[:], ones16[:], expA[:, mt, :],
                                start=(mt == 0), stop=(mt == MT - 1),
                                skip_group_check=True)

                        # A-path normalization overlaps the S-score loop
                        rcpA = small.tile([1, NF], F32, tag="rcp")
                        nc.vector.reciprocal(rcpA[:], sumA[:])
                        nc.any.tensor_scalar_mul(rcpA[:], rcpA[:], 0.5)
                        RA = small.tile([P, NF], F32, tag="RB")
                        nc.gpsimd.partition_broadcast(RA[:], rcpA[:])
                        PT = blk.tile([P, MT, NF], F32R, tag="PT")
                        for mt in range(MT):
                            nc.any.tensor_tensor(
                                PT[:, mt, :], expA[:, mt, :], RA[:], AX.mult)

                        # ---- 2a: seq scores + exp + col sums ----
                        for mt in range(MT):
                            iA = stream.tile([P, DT, P], BF16, tag="stm")
                            nc.sync.dma_start(
                                iA[:], tile_chunk(
                                    imgT_g if gather else None, imgT_h, mt))
                            psS = psAS.tile([P, NF], F32, tag="psA")
                            for ck in range(DT):
                                nc.tensor.matmul(
                                    psS[:], iA[:, ck, :], kerTn[:, ck, :],
                                    start=(ck == 0), stop=(ck == DT - 1))
                            nc.scalar.activation(
                                expS[:, mt, :], psS[:], EXP,
                                scale=maskS[:, mt:mt + 1])
                            nc.tensor.matmul(
                                sumS[:], ones16[:], expS[:, mt, :],
                                start=(mt == 0), stop=(mt == MT - 1),
                                skip_group_check=True)

                        # ---- 2b: fold S path into PT ----
                        rcpS = small.tile([1, NF], F32, tag="rcp")
                        nc.vector.reciprocal(rcpS[:], sumS[:])
                        nc.any.tensor_scalar_mul(rcpS[:], rcpS[:], 0.5)
                        RS = small.tile([P, NF], F32, tag="RB")
                        nc.gpsimd.partition_broadcast(RS[:], rcpS[:])
                        for mt in range(MT):
                            nc.any.tensor_tensor(
                                expS[:, mt, :], expS[:, mt, :], RS[:], AX.mult)
                            nc.any.tensor_tensor(
                                PT[:, mt, :], PT[:, mt, :].bitcast(F32),
                                expS[:, mt, :], AX.add)

                        # ---- 2b: yT = sum_m v x PT ----
                        yT = blk.tile([P, DT, NF], F32R, tag="yT")
                        for dt in range(DT):
                            vv = stream.tile([P, MT, P], F32R, tag="stm")
                            for h in (range(2) if gather else range(1)):
                                for mb in range(MBH):
                                    off = h * LCH + mb * PB
                                    vsrc = v_g[h][mb] if gather else v_h[mb]
                                    nc.sync.dma_start(
                                        vv[:, off:off + PB, :],
                                        vsrc[dt].bitcast(F32R).rearrange(
                                            "(o p) d -> p o d", p=P))
                            psY = psYO.tile([P, NF], F32, tag="psY")
                            for mt in range(MT):
                                nc.tensor.matmul(
                                    psY[:], vv[:, mt, :], PT[:, mt, :],
                                    start=(mt == 0), stop=(mt == MT - 1))
                            nc.any.tensor_copy(out=yT[:, dt, :], in_=psY[:])

                        # ---- 2b: outT = PwT.T @ yT + pb ----
                        for ct in range(DT):
                            pw = stream.tile([P, DT, P], F32R, tag="stm")
                            ckload(pw, PwT[ct], slice(0, P))
                            psO = psYO.tile([P, NF], F32, tag="psY")
                            for dt in range(DT):
                                nc.tensor.matmul(
                                    psO[:], pw[:, dt, :], yT[:, dt, :],
                                    start=(dt == 0), stop=(dt == DT - 1))
                            t = tmps.tile([P, NF], F32, tag="t2")
                            nc.any.tensor_scalar(
                                out=t[:], in0=psO[:], scalar1=pb[:, ct:ct + 1],
                                scalar2=None, op0=AX.add)
                            nc.sync.dma_start(
                                outT[ct * P:(ct + 1) * P, nsl], t[:])

    nc.compile()
    return nc


def prep_inputs(x, qkv_w, qkv_b, proj_w, proj_b, sp_w, sp_b, kc_w, kc_b,
                ic_w, ic_b, seq_mask, D=DIM, NQ=N // 2, NM=N, gather=True):
    """Host-side weight folding + per-core input maps."""
    DT = D // P
    MT = NM // P
    f32 = np.float32

    hd = D
    S = D
    rs_hd = 1.0 / math.sqrt(hd)
    rs_S = 1.0 / math.sqrt(S)

    Wq = qkv_w[0:D]
    Wk = qkv_w[D:2 * D]
    Wv = qkv_w[2 * D:3 * D]
    bq = qkv_b[0:D]
    bk = qkv_b[D:2 * D]
    bv = qkv_b[2 * D:3 * D]

    def strip_tile(WT, width):
        # (D, D) [c, d] -> (D//width, D, width) [tile][c][d_in]
        return np.ascontiguousarray(
            WT.reshape(D, D // width, width).transpose(1, 0, 2), dtype=f32)

    WqT = strip_tile(Wq.T.astype(np.float64), P)
    WkTs = strip_tile(Wk.T.astype(np.float64) * rs_hd, P)
    WvT = strip_tile(Wv.T.astype(np.float64), FD)
    bq_h = np.ascontiguousarray(bq.reshape(DT, P).T, dtype=f32)
    bks_h = np.ascontiguousarray((bk * rs_hd).reshape(DT, P).T, dtype=f32)

    spT = sp_w.T.astype(np.float64)
    Wker = strip_tile(spT @ kc_w.T.astype(np.float64), P)
    bker = (sp_b.astype(np.float64) @ kc_w.T.astype(np.float64)
            + kc_b.astype(np.float64))
    bker_h = np.ascontiguousarray(bker.reshape(DT, P).T.astype(f32))
    import ml_dtypes
    Wimg = strip_tile(
        (spT @ ic_w.T.astype(np.float64)) * (math.sqrt(hd) * rs_S),
        P).astype(ml_dtypes.bfloat16)
    bimg = (sp_b.astype(np.float64) @ ic_w.T.astype(np.float64)
            + ic_b.astype(np.float64)) * rs_S
    bimg_h = np.ascontiguousarray(bimg.reshape(DT, P).T.astype(f32))

    PwT = strip_tile(proj_w.T.astype(np.float64), P)
    pb_h = np.ascontiguousarray(proj_b.reshape(DT, P).T, dtype=f32)
    BV = np.ascontiguousarray(np.broadcast_to(bv, (P, D)), dtype=f32)
    maskS = np.ascontiguousarray(
        np.asarray(seq_mask)[0].reshape(MT, P).T, dtype=f32)
    ones_h = np.ones((P, 1), dtype=f32)
    ones16_h = np.ones((P, 1), dtype=ml_dtypes.bfloat16)

    shared = dict(WqT=WqT, WkTs=WkTs, WvT=WvT, Wker=Wker, Wimg=Wimg, PwT=PwT,
                  bq=bq_h, bks=bks_h, bker=bker_h, bimg=bimg_h, pb=pb_h,
                  BV=BV, maskS=maskS, ones=ones_h, ones16=ones16_h)

    in_maps = []
    for core in range(N_CORES):
        b, h = divmod(core, 2)
        xTb = np.ascontiguousarray(np.asarray(x[b]).T, dtype=f32)
        m = dict(shared)
        if not gather:
            m["xT"] = xTb[:, :NM] if NM != xTb.shape[1] else xTb
        m["xTq"] = np.ascontiguousarray(xTb[:, h * NQ:(h + 1) * NQ])
        in_maps.append(m)
    return in_maps


_NC_CACHE = {}


def kernel(**inputs):
    from concourse.bass_utils import run_bass_kernel_spmd

    key = "full"
    if key not in _NC_CACHE:
        _NC_CACHE[key] = build_nc()
    nc = _NC_CACHE[key]

    NQ = N // 2
    in_maps = prep_inputs(**inputs)
    res = run_bass_kernel_spmd(nc, in_maps, core_ids=list(range(N_CORES)))
    out = np.empty((B, N, DIM), dtype=np.float32)
    for core in range(N_CORES):
        b, h = divmod(core, 2)
        out[b, h * NQ:(h + 1) * NQ, :] = res.results[core]["outT"].T
    return out



# revision 4
# speedup vs baseline: 3.1654x; 3.1654x over previous
"""Trainium2 Bass kernel for ExactSequenceAttention (v2).

Reference math (B=4, N=2048, DIM=2048, H=1, hd=2048, S=2048):
    qkv = x @ qkv_w.T + qkv_b -> q, k, v
    attn = softmax(q @ k.T / sqrt(hd))
    ker  = (q @ sp_w.T + sp_b) @ kc_w.T + kc_b
    img  = (k @ sp_w.T + sp_b) @ ic_w.T + ic_b
    seqw = softmax((ker @ img.T / sqrt(S)) * mask)
    y    = 0.5*(attn + seqw) @ v;  out = y @ proj_w.T + proj_b

Algebraic fold (kills the img tensor entirely):
    ker @ img.T = (ker @ Wimg.T) @ k.T + outer(ker @ bimg, 1)
    with Wimg = sp_w.T@ic_w.T. Define kerW = x @ (Wq.T@Wker@Wimg.T) + bbig
    (exact), c = x @ (Wq.T@Wker@bimg) + const. Then
    seq_scores = (kerW @ k.T + outer(c, 1)) / sqrt(S)
    so BOTH score paths contract against the same k, and the whole seq
    branch costs one extra x-projection instead of ker+img+extra gather.

Sharding: 8 cores = 4 batches x 2 sequence halves. Core 2b+h owns query
rows [h*1024,(h+1)*1024) of batch b, computes k8/v for the same rows,
pair-AllGathers them (groups [0,1],[2,3],[4,5],[6,7]).

Dtypes: q/k/kerW projections and both NxN score matmuls run in fp8-e4m3
with DoubleRow perf mode (2 contraction rows/partition) — inputs are
host/device scaled into fp8 range and descaled via the exp() activation
scale. v, y=P@v and the out projection stay bf16 (fp8 there fails the
2e-2 gate; measured on CPU sim). All scores are computed transposed
(keys on partitions); softmax denominators come from a ones-row matmul;
normalization is folded into the combined weight tensor PT before a
single yT/proj chain. exp() needs no max subtraction (scores are O(1)).
"""
import math
import sys

sys.path.insert(0, "/opt/trn_rl_repo")

import numpy as np

P = 128
FD = 512        # matmul moving free dim / nb block width

DIM = 2048
B, N = 4, 2048
N_CORES = 8
GROUPS = [[0, 1], [2, 3], [4, 5], [6, 7]]

# fp8 scale plan (see module docstring):
#   x8 = fp8(x)                  (std 1.0)
#   Wq8 = fp8(32*Wq),  q8 = (psQ*(SA/32) + bq*SA)          SA=16
#   Wk8 = fp8(32*Wk),  k8 = (psK*(SK/(32*sqrt(hd))) + bk*SK/sqrt(hd)) SK=32
#   Wf8 = fp8(256*Wbig), f8 = (psF*(SF/256) + bbig*SF)     SF=16
#   psA = q8*k8' = (SA*SK/sqrt(hd)) * q.k  -> exp scale 1/(SA*SK)
#   psS likewise; c enters via DVE add of 512*c/sqrt(S).
SA, SK, SF = 16.0, 32.0, 16.0
SSC = SA * SK            # 512: score descale


def build_nc(D=DIM, NQ=N // 2, NM=N, repeat=1):
    import concourse.bacc as bacc
    import concourse.mybir as mybir
    import concourse.tile as tile
    from concourse import tile_utils
    from contextlib import ExitStack

    tile_utils.max_sbuf_usage = 204 * 1024

    F32 = mybir.dt.float32
    BF16 = mybir.dt.bfloat16
    FP8 = mybir.dt.float8e4
    AX = mybir.AluOpType
    EXP = mybir.ActivationFunctionType.Exp
    DR = mybir.MatmulPerfMode.DoubleRow

    DT = D // P          # 16 feature-dim tiles
    DB = D // FD         # 4  feature-dim blocks
    MT = NM // P         # 16 key chunks (gathered)
    NBL = NQ // FD       # 2  query blocks
    NF = FD
    NMH = NM // 2        # local (own-half) key rows
    MTH = NMH // P       # 8  local key chunks
    LCH = MT // 2        # key chunks per half

    nc = bacc.Bacc("TRN2", target_bir_lowering=False, debug=False,
                   num_devices=N_CORES)

    def din(name, shape, dt=F32):
        return nc.dram_tensor(name, list(shape), dt, kind="ExternalInput")

    x8_d = din("x8", (D, NQ), FP8)       # x[b].T own-half cols, fp8
    xbf_d = din("xbf", (D, NQ), BF16)    # same in bf16 (v path)
    Wq8 = din("Wq8", (DT, D, P), FP8)    # [dt][c_in][d_out]
    Wk8 = din("Wk8", (DT, D, P), FP8)
    Wf8 = din("Wf8", (DT, D, P), FP8)
    WvT = din("WvT", (DB, D, FD), BF16)  # [db][c_in][d_out]
    PwT = din("PwT", (DT, D, P), BF16)   # [ct][d_in][c_out]
    bqs_d = din("bqs", (P, DT))          # bq*SA
    bks_d = din("bks", (P, DT))          # bk*SK/sqrt(hd)
    bfs_d = din("bfs", (P, DT))          # bbig*SF
    pb_d = din("pb", (P, DT))
    BV_d = din("BV", (P, D))             # bv broadcast along partitions
    mask_d = din("maskS", (P, MT))       # seq_mask/SSC tiled
    cB_d = din("cB", (P, NQ))            # 512*c/sqrt(S) bcast along parts
    ones16_d = din("ones16", (P, 1), BF16)

    outT = nc.dram_tensor("outT", [D, NQ], F32, kind="ExternalOutput")

    def ckload(dst, src_2d, cols, chunks=1):
        """Load a (P, DT, w) feature-major tile in `chunks` DMAs."""
        chunks = min(chunks, DT)
        gsz = DT // chunks
        for g in range(chunks):
            nc.sync.dma_start(
                dst[:, g * gsz:(g + 1) * gsz, :],
                src_2d[g * gsz * P:(g + 1) * gsz * P, cols]
                .bitcast(dst.dtype).rearrange("(o p) w -> p o w", p=P))

    with tile.TileContext(nc) as tc:
        with ExitStack() as ctx:
            consts = ctx.enter_context(tc.tile_pool(name="consts", bufs=1))
            dram = ctx.enter_context(
                tc.tile_pool(name="dram", bufs=1, space="DRAM"))

            bqs = consts.tile([P, DT], F32)
            bks = consts.tile([P, DT], F32)
            bfs = consts.tile([P, DT], F32)
            pb = consts.tile([P, DT], F32)
            maskS = consts.tile([P, MT], F32)
            BV = consts.tile([P, D], F32)
            cB = consts.tile([P, NQ], F32)
            ones16 = consts.tile([P, 1], BF16)
            nc.sync.dma_start(bqs[:], bqs_d[:])
            nc.sync.dma_start(bks[:], bks_d[:])
            nc.sync.dma_start(bfs[:], bfs_d[:])
            nc.sync.dma_start(pb[:], pb_d[:])
            nc.sync.dma_start(maskS[:], mask_d[:])
            nc.sync.dma_start(BV[:], BV_d[:])
            nc.sync.dma_start(cB[:], cB_d[:])
            nc.sync.dma_start(ones16[:], ones16_d[:])

            # k8: [chunk(8)][p(c_in)][dt][m(128)] fp8 -- chunk-contiguous
            k8_h = dram.tile([MTH, P, DT, P], FP8)
            k8_g = dram.tile([2, MTH, P, DT, P], FP8)
            # v: [mb(2)][dt][m(512)][d(128)] bf16
            MBH, MFB = 2, FD
            v_h = dram.tile([MBH, DT, MFB, P], BF16)
            v_g = dram.tile([2, MBH, DT, MFB, P], BF16)

            def pair_gather(half_blk, gath_blk):
                nc.gpsimd.collective_compute(
                    "AllGather", mybir.AluOpType.bypass,
                    replica_groups=GROUPS,
                    ins=[half_blk[:]], outs=[gath_blk[:]])

            def fp8_pass(x8, Wsrc, out_cb, wpool, ps1):
                """x8 @ W in fp8 DoubleRow; out_cb(dt, nb, nsl, ps)."""
                for dt in range(DT):
                    w = wpool.tile([P, DT, P], FP8, tag="w", name="w")
                    ckload(w, Wsrc[dt], slice(0, P))
                    for nb in range(NBL):
                        nsl = slice(nb * NF, (nb + 1) * NF)
                        ps = ps1.tile([P, NF], F32, tag="ps", name="ps")
                        for c2 in range(DT // 2):
                            nc.tensor.matmul(
                                ps[:], w[:, 2 * c2:2 * c2 + 2, :],
                                x8[:, 2 * c2:2 * c2 + 2, nsl],
                                start=(c2 == 0), stop=(c2 == DT // 2 - 1),
                                perf_mode=DR)
                        out_cb(dt, nb, nsl, ps)

            for _rep in range(repeat):
                with ExitStack() as rep:
                    PTpool = rep.enter_context(
                        tc.tile_pool(name="PTp", bufs=1))
                    PTs = [PTpool.tile([P, MT, NF], BF16, tag=f"PT{i}",
                                       name=f"PT{i}")
                           for i in range(NBL)]

                    with ExitStack() as front:
                        qk_pool = front.enter_context(
                            tc.tile_pool(name="qkp", bufs=1))
                        qT8 = qk_pool.tile([P, DT, NQ], FP8, tag="qT8")
                        fT8 = qk_pool.tile([P, DT, NQ], FP8, tag="fT8")

                        with ExitStack() as sx:
                            xpool = sx.enter_context(
                                tc.tile_pool(name="xp", bufs=1))
                            x8 = xpool.tile([P, DT, NQ], FP8, tag="x8")
                            ckload(x8, x8_d, slice(0, NQ), chunks=8)
                            xbf = xpool.tile([P, DT, NQ], BF16, tag="xbf")
                            ckload(xbf, xbf_d, slice(0, NQ), chunks=8)

                            # ==== Stage 1a: k8 (fp8 DoubleRow) + gather ====
                            with ExitStack() as s1:
                                wpool = s1.enter_context(
                                    tc.tile_pool(name="w1a", bufs=3))
                                ps1 = s1.enter_context(
                                    tc.tile_pool(name="ps1a", bufs=4,
                                                 space="PSUM"))
                                tmps = s1.enter_context(
                                    tc.tile_pool(name="t1a", bufs=4))

                                def k_out(dt, nb, nsl, ps):
                                    t = tmps.tile([P, NF], FP8, tag="t",
                                                  name="t")
                                    nc.any.tensor_scalar(
                                        out=t[:], in0=ps[:],
                                        scalar1=SK / (32.0 * math.sqrt(D)),
                                        scalar2=bks[:, dt:dt + 1],
                                        op0=AX.mult, op1=AX.add)
                                    for mi in range(NF // P):
                                        m = nb * (NF // P) + mi
                                        nc.sync.dma_start(
                                            k8_h[m][:, dt, :],
                                            t[:, mi * P:(mi + 1) * P])

                                fp8_pass(x8, Wk8, k_out, wpool, ps1)
                                pair_gather(k8_h, k8_g)

                            # ==== Stage 1b: v (bf16) + gather ====
                            with ExitStack() as s1:
                                wpool = s1.enter_context(
                                    tc.tile_pool(name="w1b", bufs=2))
                                ps1 = s1.enter_context(
                                    tc.tile_pool(name="ps1b", bufs=4,
                                                 space="PSUM"))
                                tmps = s1.enter_context(
                                    tc.tile_pool(name="t1b", bufs=4))
                                FDP = FD // P
                                for db in range(DB):
                                    w = wpool.tile([P, DT, FD], BF16, tag="w")
                                    ckload(w, WvT[db], slice(0, FD), chunks=4)
                                    for mb in range(MBH):
                                        for mi in range(MFB // P):
                                            m = mb * (MFB // P) + mi
                                            ps = ps1.tile([P, FD], F32,
                                                          tag="ps", name="ps")
                                            for ck in range(DT):
                                                nc.tensor.matmul(
                                                    ps[:],
                                                    xbf[:, ck,
                                                        m * P:(m + 1) * P],
                                                    w[:, ck, :],
                                                    start=(ck == 0),
                                                    stop=(ck == DT - 1))
                                            t = tmps.tile([P, FD], BF16,
                                                          tag="t", name="t")
                                            nc.any.tensor_tensor(
                                                t[:], ps[:],
                                                BV[:, db * FD:(db + 1) * FD],
                                                AX.add)
                                            nc.sync.dma_start(
                                                v_h[mb][db * FDP:
                                                        (db + 1) * FDP,
                                                        mi * P:(mi + 1) * P,
                                                        :]
                                                .rearrange("o p d -> p o d"),
                                                t[:].rearrange(
                                                    "p (o d) -> p o d",
                                                    o=FDP))
                                pair_gather(v_h, v_g)

                            # ==== Stage 1c+1d: qT8, fT8 (SBUF-resident) ====
                            with ExitStack() as s1:
                                wpool = s1.enter_context(
                                    tc.tile_pool(name="w1c", bufs=3))
                                ps1 = s1.enter_context(
                                    tc.tile_pool(name="ps1c", bufs=4,
                                                 space="PSUM"))
                                for Wsrc, dst, dsc, bias in (
                                        (Wq8, qT8, SA / 32.0, bqs),
                                        (Wf8, fT8, SF / 256.0, bfs)):
                                    def qf_out(dt, nb, nsl, ps,
                                               dst=dst, dsc=dsc, bias=bias):
                                        nc.any.tensor_scalar(
                                            out=dst[:, dt, nsl], in0=ps[:],
                                            scalar1=dsc,
                                            scalar2=bias[:, dt:dt + 1],
                                            op0=AX.mult, op1=AX.add)
                                    fp8_pass(x8, Wsrc, qf_out, wpool, ps1)

                        # ==== Stage 2a: scores/softmax -> PT[nb] ====
                        with ExitStack() as s2:
                            blk = s2.enter_context(
                                tc.tile_pool(name="blk", bufs=1))
                            kres = s2.enter_context(
                                tc.tile_pool(name="kres", bufs=1))
                            small = s2.enter_context(
                                tc.tile_pool(name="small", bufs=4))
                            psAS = s2.enter_context(
                                tc.tile_pool(name="psAS", bufs=4,
                                             space="PSUM"))
                            psSums = s2.enter_context(
                                tc.tile_pool(name="psSums", bufs=1,
                                             space="PSUM"))

                            k8r = kres.tile([P, MT, DT, P], FP8, tag="k8r")

                            for nb in range(NBL):
                                nsl = slice(nb * NF, (nb + 1) * NF)
                                expA = blk.tile([P, MT, NF], BF16, tag="expA",
                                                name="expA")
                                expS = blk.tile([P, MT, NF], BF16, tag="expS",
                                                name="expS")
                                sumA = psSums.tile([1, NF], F32, tag="sumA",
                                                   name="sumA")
                                sumS = psSums.tile([1, NF], F32, tag="sumS",
                                                   name="sumS")

                                # A path (+ k8 chunk loads on first block)
                                for mt in range(MT):
                                    if nb == 0:
                                        h, l = divmod(mt, LCH)
                                        nc.sync.dma_start(
                                            k8r[:, mt, :, :], k8_g[h][l])
                                    psA = psAS.tile([P, NF], F32, tag="psA",
                                                    name="psA")
                                    for c2 in range(DT // 2):
                                        nc.tensor.matmul(
                                            psA[:],
                                            k8r[:, mt, 2 * c2:2 * c2 + 2, :],
                                            qT8[:, 2 * c2:2 * c2 + 2, nsl],
                                            start=(c2 == 0),
                                            stop=(c2 == DT // 2 - 1),
                                            perf_mode=DR)
                                    nc.scalar.activation(
                                        expA[:, mt, :], psA[:], EXP,
                                        scale=1.0 / SSC)
                                    if mt > 0:
                                        nc.tensor.matmul(
                                            sumA[:], ones16[:],
                                            expA[:, mt - 1, :],
                                            start=(mt == 1), stop=False,
                                            skip_group_check=True)
                                nc.tensor.matmul(
                                    sumA[:], ones16[:], expA[:, MT - 1, :],
                                    start=False, stop=True,
                                    skip_group_check=True)

                                # A normalization overlaps the S loop below
                                rcpA = small.tile([1, NF], F32, tag="rcp",
                                                  name="rcpA")
                                nc.vector.reciprocal(rcpA[:], sumA[:])
                                nc.any.tensor_scalar_mul(rcpA[:], rcpA[:], 0.5)
                                RA = small.tile([P, NF], F32, tag="RB",
                                                name="RA")
                                nc.gpsimd.partition_broadcast(RA[:], rcpA[:])
                                PT = PTs[nb]
                                for mt in range(MT):
                                    nc.any.tensor_tensor(
                                        PT[:, mt, :], expA[:, mt, :], RA[:],
                                        AX.mult)

                                # S path
                                for mt in range(MT):
                                    psS = psAS.tile([P, NF], F32, tag="psA",
                                                    name="psS")
                                    for c2 in range(DT // 2):
                                        nc.tensor.matmul(
                                            psS[:],
                                            k8r[:, mt, 2 * c2:2 * c2 + 2, :],
                                            fT8[:, 2 * c2:2 * c2 + 2, nsl],
                                            start=(c2 == 0),
                                            stop=(c2 == DT // 2 - 1),
                                            perf_mode=DR)
                                    nc.any.tensor_tensor(
                                        psS[:], psS[:], cB[:, nsl], AX.add)
                                    nc.scalar.activation(
                                        expS[:, mt, :], psS[:], EXP,
                                        scale=maskS[:, mt:mt + 1])
                                    if mt > 0:
                                        nc.tensor.matmul(
                                            sumS[:], ones16[:],
                                            expS[:, mt - 1, :],
                                            start=(mt == 1), stop=False,
                                            skip_group_check=True)
                                nc.tensor.matmul(
                                    sumS[:], ones16[:], expS[:, MT - 1, :],
                                    start=False, stop=True,
                                    skip_group_check=True)

                                rcpS = small.tile([1, NF], F32, tag="rcp",
                                                  name="rcpS")
                                nc.vector.reciprocal(rcpS[:], sumS[:])
                                nc.any.tensor_scalar_mul(rcpS[:], rcpS[:], 0.5)
                                RS = small.tile([P, NF], F32, tag="RB",
                                                name="RS")
                                nc.gpsimd.partition_broadcast(RS[:], rcpS[:])
                                for mt in range(MT):
                                    nc.any.tensor_tensor(
                                        expS[:, mt, :], expS[:, mt, :], RS[:],
                                        AX.mult)
                                    nc.any.tensor_tensor(
                                        PT[:, mt, :], PT[:, mt, :],
                                        expS[:, mt, :], AX.add)

                    # ==== Stage 2b: yT[nb] = sum_m v x PT ====
                    with ExitStack() as s3:
                        ypool = s3.enter_context(
                            tc.tile_pool(name="yp", bufs=1))
                        stream = s3.enter_context(
                            tc.tile_pool(name="stm2", bufs=3))
                        psY = s3.enter_context(
                            tc.tile_pool(name="psY", bufs=4, space="PSUM"))
                        tmp2 = s3.enter_context(
                            tc.tile_pool(name="t2", bufs=4))
                        yTs = [ypool.tile([P, DT, NF], BF16, tag=f"yT{i}",
                                          name=f"yT{i}")
                               for i in range(NBL)]
                        MBH, MFB = 2, FD
                        for dt in range(DT):
                            vv = stream.tile([P, MT, P], BF16, tag="stm",
                                             name="vv")
                            for hh in range(2):
                                for mb in range(MBH):
                                    off = hh * LCH + mb * (MFB // P)
                                    nc.sync.dma_start(
                                        vv[:, off:off + MFB // P, :],
                                        v_g[hh][mb][dt].rearrange(
                                            "(o p) d -> p o d", p=P))
                            for nb in range(NBL):
                                ps = psY.tile([P, NF], F32, tag="ps",
                                              name="psy")
                                for mt in range(MT):
                                    nc.tensor.matmul(
                                        ps[:], vv[:, mt, :],
                                        PTs[nb][:, mt, :],
                                        start=(mt == 0),
                                        stop=(mt == MT - 1))
                                nc.any.tensor_copy(
                                    out=yTs[nb][:, dt, :], in_=ps[:])

                        # ==== Stage 2c: outT = PwT.T @ yT + pb ====
                        for ct in range(DT):
                            pw = stream.tile([P, DT, P], BF16, tag="stm",
                                             name="pw")
                            ckload(pw, PwT[ct], slice(0, P))
                            for nb in range(NBL):
                                nsl = slice(nb * NF, (nb + 1) * NF)
                                ps = psY.tile([P, NF], F32, tag="ps",
                                              name="pso")
                                for dt in range(DT):
                                    nc.tensor.matmul(
                                        ps[:], pw[:, dt, :],
                                        yTs[nb][:, dt, :],
                                        start=(dt == 0),
                                        stop=(dt == DT - 1))
                                t = tmp2.tile([P, NF], F32, tag="t", name="t")
                                nc.any.tensor_scalar(
                                    out=t[:], in0=ps[:],
                                    scalar1=pb[:, ct:ct + 1],
                                    scalar2=None, op0=AX.add)
                                nc.sync.dma_start(
                                    outT[ct * P:(ct + 1) * P, nsl], t[:])

    nc.compile()
    return nc


def prep_inputs(x, qkv_w, qkv_b, proj_w, proj_b, sp_w, sp_b, kc_w, kc_b,
                ic_w, ic_b, seq_mask, D=DIM, NQ=N // 2, NM=N):
    """Host-side weight folding + per-core input maps."""
    import ml_dtypes
    F8 = ml_dtypes.float8_e4m3
    BF = ml_dtypes.bfloat16
    DT = D // P
    MT = NM // P
    f32 = np.float32
    f64 = np.float64

    S = D
    rs_S = 1.0 / math.sqrt(S)

    Wq = qkv_w[0:D].astype(f64)
    Wk = qkv_w[D:2 * D].astype(f64)
    Wv = qkv_w[2 * D:3 * D].astype(f64)
    bq = qkv_b[0:D].astype(f64)
    bk = qkv_b[D:2 * D].astype(f64)
    bv = qkv_b[2 * D:3 * D].astype(f64)

    def strip_tile(WT, width, dt):
        # (D, D) [c_in, d_out] -> (D//width, D, width) [tile][c_in][d_out]
        return np.ascontiguousarray(
            WT.reshape(D, D // width, width).transpose(1, 0, 2)).astype(dt)

    # seq-path folds
    Wker = sp_w.T.astype(f64) @ kc_w.T.astype(f64)
    bker = sp_b.astype(f64) @ kc_w.T.astype(f64) + kc_b.astype(f64)
    Wimg = sp_w.T.astype(f64) @ ic_w.T.astype(f64)
    bimg = sp_b.astype(f64) @ ic_w.T.astype(f64) + ic_b.astype(f64)
    Wfold = Wker @ Wimg.T                  # (D, D)
    bfold = bker @ Wimg.T                  # (D,)
    u = Wker @ bimg                        # (D,)
    cconst = float(bker @ bimg)

    WbigT = Wq.T @ Wfold                   # kerW = x @ WbigT + bbig
    bbig = bq @ Wfold + bfold
    uq = Wq.T @ u                          # c = x @ uq + (bq@u + cconst)
    cc0 = float(bq @ u) + cconst

    Wq8 = strip_tile(Wq.T * 32.0, P, F8)
    Wk8 = strip_tile(Wk.T * 32.0, P, F8)
    Wf8 = strip_tile(WbigT * 256.0, P, F8)
    WvT = strip_tile(Wv.T, FD, BF)
    PwT = strip_tile(proj_w.T.astype(f64), P, BF)

    bqs = np.ascontiguousarray((bq * SA).reshape(DT, P).T).astype(f32)
    bks = np.ascontiguousarray(
        (bk * (SK / math.sqrt(D))).reshape(DT, P).T).astype(f32)
    bfs = np.ascontiguousarray((bbig * SF).reshape(DT, P).T).astype(f32)
    pb_h = np.ascontiguousarray(
        proj_b.astype(f64).reshape(DT, P).T).astype(f32)
    BV = np.ascontiguousarray(np.broadcast_to(bv, (P, D))).astype(f32)
    maskS = np.ascontiguousarray(
        np.asarray(seq_mask, dtype=f64)[0].reshape(MT, P).T / SSC).astype(f32)
    ones16_h = np.ones((P, 1), dtype=BF)

    shared = dict(Wq8=Wq8, Wk8=Wk8, Wf8=Wf8, WvT=WvT, PwT=PwT,
                  bqs=bqs, bks=bks, bfs=bfs, pb=pb_h, BV=BV,
                  maskS=maskS, ones16=ones16_h)

    in_maps = []
    for core in range(N_CORES):
        b, h = divmod(core, 2)
        xb = np.asarray(x[b], dtype=f64)
        xT = np.ascontiguousarray(xb.T[:, h * NQ:(h + 1) * NQ])
        c = (xT.T @ uq + cc0) * (SSC * rs_S)      # [NQ]
        m = dict(shared)
        m["x8"] = xT.astype(F8)
        m["xbf"] = xT.astype(BF)
        m["cB"] = np.ascontiguousarray(
            np.broadcast_to(c.astype(f32), (P, NQ)))
        in_maps.append(m)
    return in_maps


_NC_CACHE = {}


def kernel(**inputs):
    from concourse.bass_utils import run_bass_kernel_spmd

    key = "full"
    if key not in _NC_CACHE:
        _NC_CACHE[key] = build_nc()
    nc = _NC_CACHE[key]

    NQ = N // 2
    in_maps = prep_inputs(**inputs)
    res = run_bass_kernel_spmd(nc, in_maps, core_ids=list(range(N_CORES)))
    out = np.empty((B, N, DIM), dtype=np.float32)
    for core in range(N_CORES):
        b, h = divmod(core, 2)
        out[b, h * NQ:(h + 1) * NQ, :] = res.results[core]["outT"].T
    return out
